# revision 3
# baseline (speedup 1.0000x reference)
"""Block-sparse attention (SageAttention-style mean-similarity top-k) on 8 TRN2 NeuronCores.

Sharding: 16 heads tensor-parallel across 8 cores (2 heads/core).
  - qkv weight column-sharded per core; block selection + block-sparse
    attention fully local per head
  - proj weight row-sharded: each core computes the full-shape PARTIAL product
    (+ bias on core 0 only) in bf16; the host unshard step sums the partials.

v3: interleaved phase-B pipeline to keep the PE busy:
  per iteration qb:  gathers(qb+2) | scores h0(qb) | AV h0(qb-1) |
    scores h1(qb) | AV h1(qb-1) | norm(qb-1) on DVE | PE-transpose(qb-2) |
    2 proj matmuls of the oldest pending chunk
  The transpose trails by 2 iterations so its DVE-norm dependency is always
  ready (v2 stalled the in-order PE queue ~1.4us/qb there), and proj matmuls
  are spread 2/iteration so the pj PSUM rotation never blocks on the DVE
  bias-add drain.
"""

import os
import sys

for _p in ("/opt/trn_rl_repo", "/root/.axon_site/_ro/trn_rl_repo"):
    if os.path.isdir(_p) and _p not in sys.path:
        sys.path.insert(0, _p)

import numpy as np

import concourse.bass as bass
import concourse.bacc as bacc
import concourse.tile as tile
import concourse.mybir as mybir
from concourse.bass_utils import run_bass_kernel_spmd
from concourse.library_config import ap_gather as ap_gather_lib

# problem constants
N = 4096          # sequence length
C = 1024          # model dim
H = 16            # heads
D = 64            # head dim
BLK = 128         # block size
NB = N // BLK     # 32 blocks
TOPK = 16         # int(0.5 * NB)
NCORES = 8
HPC = H // NCORES  # 2 heads per core
SCALE = D ** -0.5  # 0.125

F32 = mybir.dt.float32
BF16 = mybir.dt.bfloat16
I16 = mybir.dt.int16
U32 = mybir.dt.uint32

CHQ = 4            # query blocks per projection chunk
CHT = CHQ * BLK    # 512 tokens per chunk
NCH = NB // CHQ    # 8 chunks

_CACHE = {}


def _build():
    nc = bacc.Bacc("TRN2", target_bir_lowering=False, debug=False,
                   num_devices=NCORES)

    KC = C // 128  # 8 contraction tiles

    xT = nc.dram_tensor("xT", [C, N], F32, kind="ExternalInput")
    wqkvT = nc.dram_tensor("wqkvT", [C, 3 * 2 * D], F32, kind="ExternalInput")
    projWT = nc.dram_tensor("projWT", [2 * D, C], F32, kind="ExternalInput")
    projb = nc.dram_tensor("projb", [128, KC], F32, kind="ExternalInput")
    ident64 = nc.dram_tensor("ident64", [64, 64], F32, kind="ExternalInput")
    ident128 = nc.dram_tensor("ident128", [128, 128], F32, kind="ExternalInput")
    erep = nc.dram_tensor("erep", [16, 128], F32, kind="ExternalInput")
    out_ext = nc.dram_tensor("out", [C, N], BF16, kind="ExternalOutput")

    with tile.TileContext(nc) as tc:
        nc.gpsimd.load_library(ap_gather_lib)

        with tc.tile_pool(name="persist", bufs=1) as pp:
            # ---- weights ----
            wqkv_bf = pp.tile([128, KC, 384], BF16)
            wqk_f32 = pp.tile([128, KC, 384], F32)
            nc.sync.dma_start(
                wqk_f32[:], wqkvT.ap().rearrange("(a p) m -> p a m", p=128))
            nc.scalar.copy(wqkv_bf[:], wqk_f32[:])
            projW_bf = pp.tile([128, C], BF16)          # [c_local, j]
            projW_f32 = pp.tile([128, C], F32)
            nc.sync.dma_start(projW_f32[:], projWT.ap())
            nc.scalar.copy(projW_bf[:], projW_f32[:])
            projb_sb = pp.tile([128, KC], F32)          # bias for j-tile m in col m
            nc.sync.dma_start(projb_sb[:], projb.ap())
            id64 = pp.tile([64, 64], F32)
            nc.sync.dma_start(id64[:], ident64.ap())
            id128f = pp.tile([128, 128], F32)
            id128b = pp.tile([128, 128], BF16)
            nc.sync.dma_start(id128f[:], ident128.ap())
            nc.scalar.copy(id128b[:], id128f[:])
            erep_sb = pp.tile([16, 128], F32)
            nc.sync.dma_start(erep_sb[:], erep.ap())

            xm = pp.tile([128, KC, NB], F32)

            # ---- QKV outputs ----
            qT = pp.tile([128, N], BF16)
            kT = pp.tile([128, NB, BLK], BF16)   # contiguous == [128, N]
            v0 = pp.tile([128, NB, 66], BF16)
            v1 = pp.tile([128, NB, 66], BF16)
            nc.vector.memset(v0[:, :, 64:66], 0.0)
            nc.vector.memset(v1[:, :, 64:66], 0.0)
            nc.vector.memset(v0[:, :, 64:65], 1.0)
            nc.vector.memset(v1[:, :, 64:65], 1.0)

            # ---- phase A: x chunks -> block sums + bf16 cast + QKV ----
            with tc.tile_pool(name="xload", bufs=1) as xbp, \
                 tc.tile_pool(name="xf", bufs=3) as xp, \
                 tc.tile_pool(name="qkps", bufs=3, space="PSUM") as qp, \
                 tc.tile_pool(name="vps", bufs=3, space="PSUM") as vp:
                xbf = xbp.tile([128, KC, N], BF16)
                for nch in range(8):
                    lo, hi = nch * 512, (nch + 1) * 512
                    xf = xp.tile([128, KC, 512], F32, tag="xf", name=f"xf_{nch}")
                    nc.sync.dma_start(
                        xf[:],
                        xT.ap().rearrange("(a p) m -> p a m", p=128)[:, :, lo:hi])
                    for kc in range(KC):
                        nc.vector.tensor_reduce(
                            xm[:, kc, nch * 4:(nch + 1) * 4],
                            xf[:, kc, :].rearrange("p (b t) -> p b t", t=BLK),
                            axis=mybir.AxisListType.X, op=mybir.AluOpType.add)
                        nc.scalar.copy(xbf[:, kc, lo:hi], xf[:, kc, :])
                    for mt in (0, 1):
                        ps = qp.tile([128, 512], F32, tag="qk")
                        for kc in range(KC):
                            nc.tensor.matmul(
                                ps[:], lhsT=wqkv_bf[:, kc, mt * 128:(mt + 1) * 128],
                                rhs=xbf[:, kc, lo:hi],
                                start=(kc == 0), stop=(kc == KC - 1))
                        if mt == 0:
                            nc.scalar.copy(qT[:, lo:hi], ps[:])
                        else:
                            nc.scalar.copy(
                                kT[:].rearrange("p a b -> p (a b)")[:, lo:hi],
                                ps[:])
                    for nt in range(4 * nch, 4 * nch + 4):
                        psv = vp.tile([128, 128], F32, tag="v")
                        for kc in range(KC):
                            nc.tensor.matmul(psv[:], lhsT=xbf[:, kc, nt * 128:(nt + 1) * 128],
                                             rhs=wqkv_bf[:, kc, 256:384],
                                             start=(kc == 0), stop=(kc == KC - 1))
                        nc.vector.tensor_copy(v0[:, nt, 0:64], psv[:, 0:64])
                        nc.vector.tensor_copy(v1[:, nt, 0:64], psv[:, 64:128])

            # ---- block-mean similarity + top-k selection (f32) ----
            kidx = pp.tile([128, NB], I16)
            vidx0 = pp.tile([128, NB], I16)
            vidx1 = pp.tile([128, NB], I16)
            with tc.tile_pool(name="selps", bufs=2, space="PSUM") as sp, \
                 tc.tile_pool(name="selsb", bufs=2) as sb:
                qm_ps = sp.tile([128, NB], F32, tag="qkm")
                km_ps = sp.tile([128, NB], F32, tag="qkm")
                for kc in range(KC):
                    nc.tensor.matmul(qm_ps[:], lhsT=wqk_f32[:, kc, 0:128],
                                     rhs=xm[:, kc, :], start=(kc == 0), stop=(kc == KC - 1))
                for kc in range(KC):
                    nc.tensor.matmul(km_ps[:], lhsT=wqk_f32[:, kc, 128:256],
                                     rhs=xm[:, kc, :], start=(kc == 0), stop=(kc == KC - 1))
                qm_sb = sb.tile([128, NB], F32, tag="qm")
                km_sb = sb.tile([128, NB], F32, tag="km")
                nc.scalar.copy(qm_sb[:], qm_ps[:])
                nc.scalar.copy(km_sb[:], km_ps[:])

                sim_ps = sp.tile([64, NB], F32, tag="sim")
                for h in range(HPC):
                    nc.tensor.matmul(sim_ps[h * 32:(h + 1) * 32, :],
                                     lhsT=qm_sb[h * 64:(h + 1) * 64, :],
                                     rhs=km_sb[h * 64:(h + 1) * 64, :],
                                     start=True, stop=True)
                sim2 = sb.tile([64, NB], F32, tag="sim2")
                nc.vector.tensor_copy(sim2[:], sim_ps[:])

                vals0 = sb.tile([64, 8], F32, tag="v0")
                idx0 = sb.tile([64, 8], U32, tag="i0")
                pun = sb.tile([64, NB], F32, tag="pun")
                vals1 = sb.tile([64, 8], F32, tag="v1")
                idx1 = sb.tile([64, 8], U32, tag="i1")
                nc.vector.max(vals0[:], sim2[:])
                nc.vector.max_index(idx0[:], vals0[:], sim2[:])
                nc.vector.match_replace(out=pun[:], in_to_replace=vals0[:],
                                        in_values=sim2[:], imm_value=-1e30)
                nc.vector.max(vals1[:], pun[:])
                nc.vector.max_index(idx1[:], vals1[:], pun[:])

                idxf = sb.tile([64, TOPK], F32, tag="idxf")
                nc.vector.tensor_copy(idxf[:, 0:8], idx0[:])
                nc.vector.tensor_copy(idxf[:, 8:16], idx1[:])

                selT_ps = sp.tile([TOPK, 64], F32, tag="selT")
                nc.tensor.transpose(selT_ps[:], idxf[:], id64[:])
                selT = sb.tile([TOPK, 64], F32, tag="selTsb")
                nc.vector.tensor_copy(selT[:], selT_ps[:])

                # replicate selT rows to all 16-partition groups via one matmul:
                # rep[m, n] = selT[m % 16, n]
                rep_ps = sp.tile([128, 64], F32, tag="rep")
                nc.tensor.matmul(rep_ps[:], lhsT=erep_sb[:], rhs=selT[:],
                                 start=True, stop=True)
                nc.vector.tensor_copy(kidx[0:64, :], rep_ps[0:64, 0:32])
                nc.vector.tensor_copy(kidx[64:128, :], rep_ps[64:128, 32:64])
                nc.vector.tensor_copy(vidx0[:], rep_ps[:, 0:32])
                nc.vector.tensor_copy(vidx1[:], rep_ps[:, 32:64])

            # ---- main loop: sparse attention + chunked projection partials ----
            with tc.tile_pool(name="gather", bufs=3) as gp, \
                 tc.tile_pool(name="escore", bufs=10) as ep, \
                 tc.tile_pool(name="sps", bufs=2, space="PSUM") as spp, \
                 tc.tile_pool(name="pjps", bufs=2, space="PSUM") as jpp, \
                 tc.tile_pool(name="ops", bufs=1, space="PSUM") as opp, \
                 tc.tile_pool(name="otps", bufs=1, space="PSUM") as tpp, \
                 tc.tile_pool(name="onp", bufs=3) as onp, \
                 tc.tile_pool(name="otsb", bufs=2) as otp, \
                 tc.tile_pool(name="posb", bufs=2) as pop:

                gathers = {}
                escores = {}   # qb -> (etiles, vg0, vg1)
                onorms = {}    # qb -> onorm tile
                ot_tiles = {}  # chunk -> ot tile
                po_tiles = {}  # chunk -> po tile
                proj_q = []    # pending (chunk, m) proj ops

                def emit_gathers(qb):
                    kg = gp.tile([128, TOPK, BLK], BF16, tag="kg",
                                 name=f"kg_{qb}")
                    nc.gpsimd.ap_gather(kg[:], kT[:], kidx[:, qb:qb + 1],
                                        channels=128, num_elems=NB, d=BLK, num_idxs=TOPK)
                    vg0 = gp.tile([128, TOPK, 66], BF16, tag="vg0",
                                  name=f"vg0_{qb}")
                    nc.gpsimd.ap_gather(vg0[:], v0[:], vidx0[:, qb:qb + 1],
                                        channels=128, num_elems=NB, d=66, num_idxs=TOPK)
                    vg1 = gp.tile([128, TOPK, 66], BF16, tag="vg1",
                                  name=f"vg1_{qb}")
                    nc.gpsimd.ap_gather(vg1[:], v1[:], vidx1[:, qb:qb + 1],
                                        channels=128, num_elems=NB, d=66, num_idxs=TOPK)
                    gathers[qb] = (kg, vg0, vg1)

                def emit_scores_half(qb, half):
                    kg, vg0, vg1 = gathers[qb]
                    qcol = slice(qb * BLK, (qb + 1) * BLK)
                    s0 = spp.tile([128, 1024], F32, tag="s", name=f"s0_{qb}_{half}")
                    s1 = spp.tile([128, 1024], F32, tag="s", name=f"s1_{qb}_{half}")
                    for jj in range(8):
                        j = half * 8 + jj
                        nc.tensor.matmul(s0[:, jj * 128:(jj + 1) * 128],
                                         lhsT=kg[0:64, j, :], rhs=qT[0:64, qcol],
                                         start=True, stop=True)
                        nc.tensor.matmul(s1[:, jj * 128:(jj + 1) * 128],
                                         lhsT=kg[64:128, j, :], rhs=qT[64:128, qcol],
                                         start=True, stop=True)
                    e0 = ep.tile([128, 1024], BF16, tag="e", name=f"e0_{qb}_{half}")
                    e1 = ep.tile([128, 1024], BF16, tag="e", name=f"e1_{qb}_{half}")
                    nc.scalar.activation(e0[:], s0[:],
                                         mybir.ActivationFunctionType.Exp, scale=SCALE)
                    nc.scalar.activation(e1[:], s1[:],
                                         mybir.ActivationFunctionType.Exp, scale=SCALE)
                    if half == 0:
                        escores[qb] = [[e0, None], [e1, None]]
                    else:
                        escores[qb][0][1] = e0
                        escores[qb][1][1] = e1

                def emit_av(qb, h):
                    etiles = escores[qb]
                    _, vg0, vg1 = gathers[qb]
                    if h == 0:
                        o_ps = opp.tile([128, 2, 66], F32, tag="o",
                                        name=f"o_{qb}")
                        escores[qb].append(o_ps)
                    o_ps = escores[qb][2]
                    vg = vg0 if h == 0 else vg1
                    for j in range(TOPK):
                        nc.tensor.matmul(o_ps[:, h, 0:65],
                                         lhsT=etiles[h][j // 8][:, (j % 8) * 128:(j % 8 + 1) * 128],
                                         rhs=vg[:, j, 0:65],
                                         start=(j == 0), stop=(j == TOPK - 1))

                def emit_norm(qb):
                    o_ps = escores[qb][2]
                    onorm = onp.tile([128, 128], BF16, tag="onorm",
                                     name=f"on_{qb}")
                    for h in (0, 1):
                        rec = onp.tile([128, 1], F32, tag="rec", name=f"r_{qb}_{h}")
                        nc.vector.reciprocal(rec[:], o_ps[:, h, 64:65])
                        nc.vector.tensor_scalar(onorm[:, h * D:(h + 1) * D],
                                                o_ps[:, h, 0:D], rec[:], None,
                                                op0=mybir.AluOpType.mult)
                    onorms[qb] = onorm
                    del escores[qb]
                    del gathers[qb]

                def emit_transpose(qb):
                    onorm = onorms.pop(qb)
                    otps = tpp.tile([128, 128], BF16, tag="ot",
                                    name=f"otp_{qb}")
                    nc.tensor.transpose(otps[:], onorm[:], id128b[:])
                    c = qb // CHQ
                    if c not in ot_tiles:
                        ot_tiles[c] = otp.tile([128, CHT], BF16, tag="ot",
                                               name=f"ot_{c}")
                    nc.vector.tensor_copy(
                        ot_tiles[c][:, (qb % CHQ) * 128:(qb % CHQ + 1) * 128],
                        otps[:])
                    if qb % CHQ == CHQ - 1:
                        po_tiles[c] = pop.tile([128, KC, CHT], BF16, tag="po",
                                               name=f"po_{c}")
                        for m in range(KC):
                            proj_q.append((c, m))

                def emit_proj_some(k):
                    for _ in range(k):
                        if not proj_q:
                            return
                        c, m = proj_q.pop(0)
                        pj = jpp.tile([128, CHT], F32, tag="pj", name=f"pj_{c}_{m}")
                        nc.tensor.matmul(pj[:],
                                         lhsT=projW_bf[:, m * 128:(m + 1) * 128],
                                         rhs=ot_tiles[c][:], start=True, stop=True)
                        nc.vector.tensor_scalar(po_tiles[c][:, m, :], pj[:],
                                                projb_sb[:, m:m + 1], None,
                                                op0=mybir.AluOpType.add)
                        if m == KC - 1:
                            nc.sync.dma_start(
                                out_ext.ap()[:, c * CHT:(c + 1) * CHT]
                                .rearrange("(a p) m -> p a m", p=128),
                                po_tiles.pop(c)[:])
                            ot_tiles.pop(c)

                # ---- software-pipelined main loop ----
                emit_gathers(0)
                emit_gathers(1)
                for qb in range(NB):
                    if qb + 2 < NB:
                        emit_gathers(qb + 2)
                    emit_scores_half(qb, 0)
                    if qb > 0:
                        emit_av(qb - 1, 0)
                    emit_scores_half(qb, 1)
                    if qb > 0:
                        emit_av(qb - 1, 1)
                        emit_norm(qb - 1)
                    if qb > 1:
                        emit_transpose(qb - 2)
                    emit_proj_some(2)
                # epilogue
                emit_av(NB - 1, 0)
                emit_av(NB - 1, 1)
                emit_norm(NB - 1)
                emit_transpose(NB - 2)
                emit_transpose(NB - 1)
                emit_proj_some(len(proj_q))

    nc.compile()
    return nc


def _prep_inputs(x, qkv_w, proj_w, proj_b):
    x = np.asarray(x, dtype=np.float32)
    qkv_w = np.asarray(qkv_w, dtype=np.float32)
    proj_w = np.asarray(proj_w, dtype=np.float32)
    proj_b = np.asarray(proj_b, dtype=np.float32)

    xT = np.ascontiguousarray(x[0].T)                      # [C, N]
    ident64 = np.eye(64, dtype=np.float32)
    ident128 = np.eye(128, dtype=np.float32)
    erep = (np.arange(128)[None, :] % 16 == np.arange(16)[:, None]).astype(np.float32)
    zero_b = np.zeros((128, 8), dtype=np.float32)
    in_maps = []
    for i in range(NCORES):
        h0 = HPC * i
        rows = []
        for part in range(3):                              # q, k, v row groups
            base = part * C + h0 * D
            rows.append(qkv_w[base:base + HPC * D, :])
        wqkv = np.concatenate(rows, axis=0)                # [384, C]
        cslice = slice(i * 2 * D, (i + 1) * 2 * D)
        in_maps.append({
            "xT": xT,
            "wqkvT": np.ascontiguousarray(wqkv.T),         # [C, 384]
            "projWT": np.ascontiguousarray(proj_w[:, cslice].T),
            "projb": (np.ascontiguousarray(proj_b.reshape(8, 128).T)
                      if i == 0 else zero_b),
            "ident64": ident64,
            "ident128": ident128,
            "erep": erep,
        })
    return in_maps


def kernel(x, qkv_w, proj_w, proj_b, _trace=False):
    if "nc" not in _CACHE:
        _CACHE["nc"] = _build()
    nc = _CACHE["nc"]
    in_maps = _prep_inputs(x, qkv_w, proj_w, proj_b)
    res = run_bass_kernel_spmd(nc, in_maps, core_ids=list(range(NCORES)),
                               trace=_trace)
    outT = res.results[0]["out"].astype(np.float32)
    for i in range(1, NCORES):
        outT += res.results[i]["out"].astype(np.float32)
    out = np.ascontiguousarray(outT.T).reshape(1, N, C).astype(np.float32)
    if _trace:
        _CACHE["last_exec_time_ns"] = res.exec_time_ns
        _CACHE["last_results"] = res
    return out


# revision 7
# speedup vs baseline: 1.2334x; 1.2334x over previous
"""Block-sparse attention (SageAttention-style mean-similarity top-k) on 8 TRN2 NeuronCores.

Sharding: 16 heads tensor-parallel across 8 cores (2 heads/core).
  - qkv weight column-sharded per core; block selection + block-sparse
    attention fully local per head
  - proj weight row-sharded: each core computes the full-shape PARTIAL product
    (+ bias on core 0 only) in fp16; the host unshard step sums the partials.

v4 (from the v1 structure, which measured best):
  - fp16 end-to-end: x uploaded fp16 (halves the 17MB x DMA; CPU sim shows the
    f32-sum-of-fp16 selection keeps all top-k picks identical), weights host-cast
    to fp16, all PE work in fp16 (same speed as bf16, more mantissa), partial
    outputs fp16. Kills the 27us ACT x-cast entirely.
  - proj pj PSUM decoupled from the scores pool (v1 coupled them and the
    proj/scores rotation stalled both): s bufs=3 (6 banks) + o_ps bufs=1 +
    pj bufs=1, CHQ=4, proj matmuls spread 2 per iteration.
  - x loaded with one DMA per 512-token chunk; obounce writes on the scalar
    DGE queue, po/out on the vector queue, x/weights/ot on sync.
"""

import os
import sys

for _p in ("/opt/trn_rl_repo", "/root/.axon_site/_ro/trn_rl_repo"):
    if os.path.isdir(_p) and _p not in sys.path:
        sys.path.insert(0, _p)

import numpy as np

import concourse.bass as bass
import concourse.bacc as bacc
import concourse.tile as tile
import concourse.mybir as mybir
from concourse.bass_utils import run_bass_kernel_spmd
from concourse.library_config import ap_gather as ap_gather_lib

# problem constants
N = 4096          # sequence length
C = 1024          # model dim
H = 16            # heads
D = 64            # head dim
BLK = 128         # block size
NB = N // BLK     # 32 blocks
TOPK = 16         # int(0.5 * NB)
NCORES = 8
HPC = H // NCORES  # 2 heads per core
SCALE = D ** -0.5  # 0.125

F32 = mybir.dt.float32
F16 = mybir.dt.float16
I16 = mybir.dt.int16
U32 = mybir.dt.uint32

CHQ = 4            # query blocks per projection chunk
CHT = CHQ * BLK    # 512 tokens per chunk

_CACHE = {}


def _build():
    nc = bacc.Bacc("TRN2", target_bir_lowering=False, debug=False,
                   num_devices=NCORES)

    KC = C // 128  # 8 contraction tiles

    xT = nc.dram_tensor("xT", [C, N], F16, kind="ExternalInput")
    wqkvT = nc.dram_tensor("wqkvT", [C, 3 * 2 * D], F16, kind="ExternalInput")
    wqkT32 = nc.dram_tensor("wqkT32", [C, 2 * 2 * D], F32, kind="ExternalInput")
    projWT = nc.dram_tensor("projWT", [2 * D, C], F16, kind="ExternalInput")
    projb = nc.dram_tensor("projb", [128, KC], F32, kind="ExternalInput")
    ident64 = nc.dram_tensor("ident64", [64, 64], F32, kind="ExternalInput")
    erep = nc.dram_tensor("erep", [16, 128], F32, kind="ExternalInput")
    out_ext = nc.dram_tensor("out", [C, N], F16, kind="ExternalOutput")

    obounce = nc.dram_tensor("obounce", [N, 2 * D], F16)

    with tile.TileContext(nc) as tc:
        nc.gpsimd.load_library(ap_gather_lib)

        with tc.tile_pool(name="persist", bufs=1) as pp:
            # ---- QKV outputs ----
            qT = pp.tile([128, N], F16)
            kT = pp.tile([128, NB, BLK], F16)   # contiguous == [128, N]
            v0 = pp.tile([128, NB, 66], F16)
            v1 = pp.tile([128, NB, 66], F16)
            nc.vector.memset(v0[:, :, 64:66], 0.0)
            nc.vector.memset(v1[:, :, 64:66], 0.0)
            nc.vector.memset(v0[:, :, 64:65], 1.0)
            nc.vector.memset(v1[:, :, 64:65], 1.0)

            xm = pp.tile([128, KC, NB], F32)

            # ---- weights ----
            wqkv_h = pp.tile([128, KC, 384], F16)
            nc.sync.dma_start(
                wqkv_h[:], wqkvT.ap().rearrange("(a p) m -> p a m", p=128))
            wqk_f32 = pp.tile([128, KC, 256], F32)
            nc.gpsimd.dma_start(
                wqk_f32[:], wqkT32.ap().rearrange("(a p) m -> p a m", p=128))
            projW_h = pp.tile([128, C], F16)          # [c_local, j]
            nc.gpsimd.dma_start(projW_h[:], projWT.ap())
            projb_sb = pp.tile([128, KC], F32)        # bias for j-tile m in col m
            nc.gpsimd.dma_start(projb_sb[:], projb.ap())
            id64 = pp.tile([64, 64], F32)
            nc.gpsimd.dma_start(id64[:], ident64.ap())
            erep_sb = pp.tile([16, 128], F32)
            nc.gpsimd.dma_start(erep_sb[:], erep.ap())

            kidx = pp.tile([128, NB], I16)
            vidx0 = pp.tile([128, NB], I16)
            vidx1 = pp.tile([128, NB], I16)

            # ---- phase A: x chunks -> block sums + QKV (all fp16) ----
            with tc.tile_pool(name="xf", bufs=3) as xp, \
                 tc.tile_pool(name="qkps", bufs=3, space="PSUM") as qp, \
                 tc.tile_pool(name="vps", bufs=3, space="PSUM") as vp:
                for nch in range(8):
                    lo, hi = nch * 512, (nch + 1) * 512
                    xf = xp.tile([128, KC, 512], F16, tag="xf", name=f"xf_{nch}")
                    nc.sync.dma_start(
                        xf[:],
                        xT.ap().rearrange("(a p) m -> p a m", p=128)[:, :, lo:hi])
                    for kc in range(KC):
                        nc.vector.tensor_reduce(
                            xm[:, kc, nch * 4:(nch + 1) * 4],
                            xf[:, kc, :].rearrange("p (b t) -> p b t", t=BLK),
                            axis=mybir.AxisListType.X, op=mybir.AluOpType.add)
                    for mt in (0, 1):
                        ps = qp.tile([128, 512], F32, tag="qk")
                        for kc in range(KC):
                            nc.tensor.matmul(
                                ps[:], lhsT=wqkv_h[:, kc, mt * 128:(mt + 1) * 128],
                                rhs=xf[:, kc, :],
                                start=(kc == 0), stop=(kc == KC - 1))
                        if mt == 0:
                            nc.scalar.copy(qT[:, lo:hi], ps[:])
                        else:
                            nc.scalar.copy(
                                kT[:].rearrange("p a b -> p (a b)")[:, lo:hi],
                                ps[:])
                    for j4 in range(4):
                        psv = vp.tile([128, 128], F32, tag="v")
                        nt = 4 * nch + j4
                        for kc in range(KC):
                            nc.tensor.matmul(psv[:], lhsT=xf[:, kc, j4 * 128:(j4 + 1) * 128],
                                             rhs=wqkv_h[:, kc, 256:384],
                                             start=(kc == 0), stop=(kc == KC - 1))
                        nc.vector.tensor_copy(v0[:, nt, 0:64], psv[:, 0:64])
                        nc.vector.tensor_copy(v1[:, nt, 0:64], psv[:, 64:128])

            # ---- block-mean similarity + top-k selection (f32) ----
            with tc.tile_pool(name="selps", bufs=1, space="PSUM") as sp, \
                 tc.tile_pool(name="selsb", bufs=2) as sb:
                qm_ps = sp.tile([128, NB], F32, tag="qm")
                km_ps = sp.tile([128, NB], F32, tag="km")
                for kc in range(KC):
                    nc.tensor.matmul(qm_ps[:], lhsT=wqk_f32[:, kc, 0:128],
                                     rhs=xm[:, kc, :], start=(kc == 0), stop=(kc == KC - 1))
                for kc in range(KC):
                    nc.tensor.matmul(km_ps[:], lhsT=wqk_f32[:, kc, 128:256],
                                     rhs=xm[:, kc, :], start=(kc == 0), stop=(kc == KC - 1))
                qm_sb = sb.tile([128, NB], F32, tag="qm")
                km_sb = sb.tile([128, NB], F32, tag="km")
                nc.scalar.copy(qm_sb[:], qm_ps[:])
                nc.scalar.copy(km_sb[:], km_ps[:])

                sim_ps = sp.tile([64, NB], F32, tag="sim")
                for h in range(HPC):
                    nc.tensor.matmul(sim_ps[h * 32:(h + 1) * 32, :],
                                     lhsT=qm_sb[h * 64:(h + 1) * 64, :],
                                     rhs=km_sb[h * 64:(h + 1) * 64, :],
                                     start=True, stop=True)
                sim2 = sb.tile([64, NB], F32, tag="sim2")
                nc.vector.tensor_copy(sim2[:], sim_ps[:])

                vals0 = sb.tile([64, 8], F32, tag="v0")
                idx0 = sb.tile([64, 8], U32, tag="i0")
                pun = sb.tile([64, NB], F32, tag="pun")
                vals1 = sb.tile([64, 8], F32, tag="v1")
                idx1 = sb.tile([64, 8], U32, tag="i1")
                nc.vector.max(vals0[:], sim2[:])
                nc.vector.max_index(idx0[:], vals0[:], sim2[:])
                nc.vector.match_replace(out=pun[:], in_to_replace=vals0[:],
                                        in_values=sim2[:], imm_value=-1e30)
                nc.vector.max(vals1[:], pun[:])
                nc.vector.max_index(idx1[:], vals1[:], pun[:])

                idxf = sb.tile([64, TOPK], F32, tag="idxf")
                nc.vector.tensor_copy(idxf[:, 0:8], idx0[:])
                nc.vector.tensor_copy(idxf[:, 8:16], idx1[:])

                selT_ps = sp.tile([TOPK, 64], F32, tag="selT")
                nc.tensor.transpose(selT_ps[:], idxf[:], id64[:])
                selT = sb.tile([TOPK, 64], F32, tag="selTsb")
                nc.vector.tensor_copy(selT[:], selT_ps[:])

                # replicate selT rows to all 16-partition groups via one matmul
                rep_ps = sp.tile([128, 64], F32, tag="rep")
                nc.tensor.matmul(rep_ps[:], lhsT=erep_sb[:], rhs=selT[:],
                                 start=True, stop=True)
                nc.vector.tensor_copy(kidx[0:64, :], rep_ps[0:64, 0:32])
                nc.vector.tensor_copy(kidx[64:128, :], rep_ps[64:128, 32:64])
                nc.vector.tensor_copy(vidx0[:], rep_ps[:, 0:32])
                nc.vector.tensor_copy(vidx1[:], rep_ps[:, 32:64])

            # ---- main loop: sparse attention + chunked projection partials ----
            with tc.tile_pool(name="gather", bufs=3) as gp, \
                 tc.tile_pool(name="escore", bufs=12) as ep, \
                 tc.tile_pool(name="sps", bufs=3, space="PSUM") as spp, \
                 tc.tile_pool(name="pjps", bufs=1, space="PSUM") as jpp, \
                 tc.tile_pool(name="ops", bufs=1, space="PSUM") as opp, \
                 tc.tile_pool(name="onp", bufs=3) as onp, \
                 tc.tile_pool(name="otsb", bufs=2) as otp, \
                 tc.tile_pool(name="posb", bufs=2) as pop:

                gathers = {}
                escores = {}
                ot_tiles = {}
                po_tiles = {}
                proj_q = []

                def emit_gathers(qb):
                    kg = gp.tile([128, TOPK, BLK], F16, tag="kg",
                                 name=f"kg_{qb}")
                    nc.gpsimd.ap_gather(kg[:], kT[:], kidx[:, qb:qb + 1],
                                        channels=128, num_elems=NB, d=BLK, num_idxs=TOPK)
                    vg0 = gp.tile([128, TOPK, 66], F16, tag="vg0",
                                  name=f"vg0_{qb}")
                    nc.gpsimd.ap_gather(vg0[:], v0[:], vidx0[:, qb:qb + 1],
                                        channels=128, num_elems=NB, d=66, num_idxs=TOPK)
                    vg1 = gp.tile([128, TOPK, 66], F16, tag="vg1",
                                  name=f"vg1_{qb}")
                    nc.gpsimd.ap_gather(vg1[:], v1[:], vidx1[:, qb:qb + 1],
                                        channels=128, num_elems=NB, d=66, num_idxs=TOPK)
                    gathers[qb] = (kg, vg0, vg1)

                def emit_scores_half(qb, half):
                    kg, _, _ = gathers[qb]
                    qcol = slice(qb * BLK, (qb + 1) * BLK)
                    s0 = spp.tile([128, 1024], F32, tag="s", name=f"s0_{qb}_{half}")
                    s1 = spp.tile([128, 1024], F32, tag="s", name=f"s1_{qb}_{half}")
                    for jj in range(8):
                        j = half * 8 + jj
                        nc.tensor.matmul(s0[:, jj * 128:(jj + 1) * 128],
                                         lhsT=kg[0:64, j, :], rhs=qT[0:64, qcol],
                                         start=True, stop=True)
                        nc.tensor.matmul(s1[:, jj * 128:(jj + 1) * 128],
                                         lhsT=kg[64:128, j, :], rhs=qT[64:128, qcol],
                                         start=True, stop=True)
                    e0 = ep.tile([128, 1024], F16, tag="e", name=f"e0_{qb}_{half}")
                    e1 = ep.tile([128, 1024], F16, tag="e", name=f"e1_{qb}_{half}")
                    nc.scalar.activation(e0[:], s0[:],
                                         mybir.ActivationFunctionType.Exp, scale=SCALE)
                    nc.scalar.activation(e1[:], s1[:],
                                         mybir.ActivationFunctionType.Exp, scale=SCALE)
                    if half == 0:
                        escores[qb] = [[e0, None], [e1, None]]
                    else:
                        escores[qb][0][1] = e0
                        escores[qb][1][1] = e1

                def emit_av(qb, h):
                    etiles = escores[qb]
                    _, vg0, vg1 = gathers[qb]
                    if h == 0:
                        o_ps = opp.tile([128, 2, 66], F32, tag="o",
                                        name=f"o_{qb}")
                        escores[qb].append(o_ps)
                    o_ps = escores[qb][2]
                    vg = vg0 if h == 0 else vg1
                    for j in range(TOPK):
                        nc.tensor.matmul(o_ps[:, h, 0:65],
                                         lhsT=etiles[h][j // 8][:, (j % 8) * 128:(j % 8 + 1) * 128],
                                         rhs=vg[:, j, 0:65],
                                         start=(j == 0), stop=(j == TOPK - 1))

                def emit_norm(qb):
                    o_ps = escores[qb][2]
                    onorm = onp.tile([128, 2 * D], F16, tag="onorm",
                                     name=f"on_{qb}")
                    for h in (0, 1):
                        rec = onp.tile([128, 1], F32, tag="rec", name=f"r_{qb}_{h}")
                        nc.vector.reciprocal(rec[:], o_ps[:, h, 64:65])
                        nc.vector.tensor_scalar(onorm[:, h * D:(h + 1) * D],
                                                o_ps[:, h, 0:D], rec[:], None,
                                                op0=mybir.AluOpType.mult)
                    nc.scalar.dma_start(obounce.ap()[qb * BLK:(qb + 1) * BLK, :],
                                        onorm[:])
                    del escores[qb]
                    del gathers[qb]
                    if qb % CHQ == CHQ - 1:
                        c = qb // CHQ
                        ot = otp.tile([128, CHT], F16, tag="ot", name=f"ot_{c}")
                        nc.sync.dma_start_transpose(
                            ot[:], obounce.ap()[c * CHT:(c + 1) * CHT, :])
                        ot_tiles[c] = ot
                        po_tiles[c] = pop.tile([128, KC, CHT], F16, tag="po",
                                               name=f"po_{c}")
                        for m in range(KC):
                            proj_q.append((c, m))

                def emit_proj_some(k):
                    for _ in range(k):
                        if not proj_q:
                            return
                        c, m = proj_q.pop(0)
                        pj = jpp.tile([128, CHT], F32, tag="pj", name=f"pj_{c}_{m}")
                        nc.tensor.matmul(pj[:],
                                         lhsT=projW_h[:, m * 128:(m + 1) * 128],
                                         rhs=ot_tiles[c][:], start=True, stop=True)
                        nc.vector.tensor_scalar(po_tiles[c][:, m, :], pj[:],
                                                projb_sb[:, m:m + 1], None,
                                                op0=mybir.AluOpType.add)
                        if m == KC - 1:
                            nc.sync.dma_start(
                                out_ext.ap()[:, c * CHT:(c + 1) * CHT]
                                .rearrange("(a p) m -> p a m", p=128),
                                po_tiles.pop(c)[:])
                            ot_tiles.pop(c)

                # ---- software-pipelined main loop ----
                emit_gathers(0)
                emit_gathers(1)
                for qb in range(NB):
                    if qb + 2 < NB:
                        emit_gathers(qb + 2)
                    emit_proj_some(1)
                    emit_scores_half(qb, 0)
                    if qb > 0:
                        emit_av(qb - 1, 0)
                    emit_proj_some(1)
                    emit_scores_half(qb, 1)
                    if qb > 0:
                        emit_av(qb - 1, 1)
                        emit_norm(qb - 1)
                # epilogue
                emit_av(NB - 1, 0)
                emit_av(NB - 1, 1)
                emit_norm(NB - 1)
                emit_proj_some(len(proj_q))

    nc.compile()
    return nc


def _prep_inputs(x, qkv_w, proj_w, proj_b):
    x = np.asarray(x, dtype=np.float32)
    qkv_w = np.asarray(qkv_w, dtype=np.float32)
    proj_w = np.asarray(proj_w, dtype=np.float32)
    proj_b = np.asarray(proj_b, dtype=np.float32)

    xT = np.ascontiguousarray(x[0].T).astype(np.float16)   # [C, N] fp16
    ident64 = np.eye(64, dtype=np.float32)
    erep = (np.arange(128)[None, :] % 16 == np.arange(16)[:, None]).astype(np.float32)
    zero_b = np.zeros((128, 8), dtype=np.float32)
    in_maps = []
    for i in range(NCORES):
        h0 = HPC * i
        rows = []
        for part in range(3):                              # q, k, v row groups
            base = part * C + h0 * D
            rows.append(qkv_w[base:base + HPC * D, :])
        wqkv = np.concatenate(rows, axis=0)                # [384, C]
        wqkvT_np = np.ascontiguousarray(wqkv.T)            # [C, 384]
        cslice = slice(i * 2 * D, (i + 1) * 2 * D)
        in_maps.append({
            "xT": xT,
            "wqkvT": wqkvT_np.astype(np.float16),
            "wqkT32": np.ascontiguousarray(wqkvT_np[:, 0:256]),
            "projWT": np.ascontiguousarray(proj_w[:, cslice].T).astype(np.float16),
            "projb": (np.ascontiguousarray(proj_b.reshape(8, 128).T)
                      if i == 0 else zero_b),
            "ident64": ident64,
            "erep": erep,
        })
    return in_maps


def kernel(x, qkv_w, proj_w, proj_b, _trace=False):
    if "nc" not in _CACHE:
        _CACHE["nc"] = _build()
    nc = _CACHE["nc"]
    in_maps = _prep_inputs(x, qkv_w, proj_w, proj_b)
    res = run_bass_kernel_spmd(nc, in_maps, core_ids=list(range(NCORES)),
                               trace=_trace)
    outT = res.results[0]["out"].astype(np.float32)
    for i in range(1, NCORES):
        outT += res.results[i]["out"].astype(np.float32)
    out = np.ascontiguousarray(outT.T).reshape(1, N, C).astype(np.float32)
    if _trace:
        _CACHE["last_exec_time_ns"] = res.exec_time_ns
        _CACHE["last_results"] = res
    return out


# revision 9
# speedup vs baseline: 1.2899x; 1.0458x over previous
"""Block-sparse attention (SageAttention-style mean-similarity top-k) on 8 TRN2 NeuronCores.

Sharding: 16 heads tensor-parallel across 8 cores (2 heads/core).
  - qkv weight column-sharded per core; block selection + block-sparse
    attention fully local per head
  - proj weight row-sharded: each core computes the full-shape PARTIAL product
    (+ bias on core 0 only) in fp16; the host unshard step sums the partials.

v4 (from the v1 structure, which measured best):
  - fp16 end-to-end: x uploaded fp16 (halves the 17MB x DMA; CPU sim shows the
    f32-sum-of-fp16 selection keeps all top-k picks identical), weights host-cast
    to fp16, all PE work in fp16 (same speed as bf16, more mantissa), partial
    outputs fp16. Kills the 27us ACT x-cast entirely.
  - proj pj PSUM decoupled from the scores pool (v1 coupled them and the
    proj/scores rotation stalled both): s bufs=3 (6 banks) + o_ps bufs=1 +
    pj bufs=1, CHQ=4, proj matmuls spread 2 per iteration.
  - x loaded with one DMA per 512-token chunk; obounce writes on the scalar
    DGE queue, po/out on the vector queue, x/weights/ot on sync.
"""

import os
import sys

for _p in ("/opt/trn_rl_repo", "/root/.axon_site/_ro/trn_rl_repo"):
    if os.path.isdir(_p) and _p not in sys.path:
        sys.path.insert(0, _p)

import numpy as np

import concourse.bass as bass
import concourse.bacc as bacc
import concourse.tile as tile
import concourse.mybir as mybir
from concourse.bass_utils import run_bass_kernel_spmd
from concourse.library_config import ap_gather as ap_gather_lib

# problem constants
N = 4096          # sequence length
C = 1024          # model dim
H = 16            # heads
D = 64            # head dim
BLK = 128         # block size
NB = N // BLK     # 32 blocks
TOPK = 16         # int(0.5 * NB)
NCORES = 8
HPC = H // NCORES  # 2 heads per core
SCALE = D ** -0.5  # 0.125

F32 = mybir.dt.float32
F16 = mybir.dt.float16
I16 = mybir.dt.int16
U32 = mybir.dt.uint32

CHQ = 4            # query blocks per projection chunk
CHT = CHQ * BLK    # 512 tokens per chunk

_CACHE = {}


def _build():
    nc = bacc.Bacc("TRN2", target_bir_lowering=False, debug=False,
                   num_devices=NCORES)

    KC = C // 128  # 8 contraction tiles

    xT = nc.dram_tensor("xT", [C, N], F16, kind="ExternalInput")
    wqkvT = nc.dram_tensor("wqkvT", [C, 3 * 2 * D], F16, kind="ExternalInput")
    wqkT32 = nc.dram_tensor("wqkT32", [C, 2 * 2 * D], F32, kind="ExternalInput")
    projWT = nc.dram_tensor("projWT", [2 * D, C], F16, kind="ExternalInput")
    projb = nc.dram_tensor("projb", [128, KC], F32, kind="ExternalInput")
    ident64 = nc.dram_tensor("ident64", [64, 64], F32, kind="ExternalInput")
    erep = nc.dram_tensor("erep", [16, 128], F32, kind="ExternalInput")
    out_ext = nc.dram_tensor("out", [C, N], F16, kind="ExternalOutput")

    obounce = nc.dram_tensor("obounce", [N, 2 * D], F16)

    with tile.TileContext(nc) as tc:
        nc.gpsimd.load_library(ap_gather_lib)

        with tc.tile_pool(name="persist", bufs=1) as pp:
            # ---- QKV outputs ----
            qT = pp.tile([128, N], F16)
            kT = pp.tile([128, NB, BLK], F16)   # contiguous == [128, N]
            v0 = pp.tile([128, NB, 66], F16)
            v1 = pp.tile([128, NB, 66], F16)
            nc.vector.memset(v0[:, :, 64:66], 0.0)
            nc.vector.memset(v1[:, :, 64:66], 0.0)
            nc.vector.memset(v0[:, :, 64:65], 1.0)
            nc.vector.memset(v1[:, :, 64:65], 1.0)

            xm = pp.tile([128, KC, NB], F32)

            # ---- weights ----
            wqkv_h = pp.tile([128, KC, 384], F16)
            nc.sync.dma_start(
                wqkv_h[:], wqkvT.ap().rearrange("(a p) m -> p a m", p=128))
            wqk_f32 = pp.tile([128, KC, 256], F32)
            nc.gpsimd.dma_start(
                wqk_f32[:], wqkT32.ap().rearrange("(a p) m -> p a m", p=128))
            projW_h = pp.tile([128, C], F16)          # [c_local, j]
            nc.gpsimd.dma_start(projW_h[:], projWT.ap())
            projb_sb = pp.tile([128, KC], F32)        # bias for j-tile m in col m
            nc.gpsimd.dma_start(projb_sb[:], projb.ap())
            id64 = pp.tile([64, 64], F32)
            nc.gpsimd.dma_start(id64[:], ident64.ap())
            erep_sb = pp.tile([16, 128], F32)
            nc.gpsimd.dma_start(erep_sb[:], erep.ap())

            kidx = pp.tile([128, NB], I16)
            vidx0 = pp.tile([128, NB], I16)
            vidx1 = pp.tile([128, NB], I16)

            # ---- phase A: x chunks -> block sums + QKV (all fp16) ----
            with tc.tile_pool(name="xf", bufs=3) as xp, \
                 tc.tile_pool(name="qkps", bufs=3, space="PSUM") as qp, \
                 tc.tile_pool(name="vps", bufs=3, space="PSUM") as vp:
                for nch in range(8):
                    lo, hi = nch * 512, (nch + 1) * 512
                    xf = xp.tile([128, KC, 512], F16, tag="xf", name=f"xf_{nch}")
                    nc.sync.dma_start(
                        xf[:],
                        xT.ap().rearrange("(a p) m -> p a m", p=128)[:, :, lo:hi])
                    for kc in range(KC):
                        nc.vector.tensor_reduce(
                            xm[:, kc, nch * 4:(nch + 1) * 4],
                            xf[:, kc, :].rearrange("p (b t) -> p b t", t=BLK),
                            axis=mybir.AxisListType.X, op=mybir.AluOpType.add)
                    for mt in (0, 1):
                        ps = qp.tile([128, 512], F32, tag="qk")
                        for kc in range(KC):
                            nc.tensor.matmul(
                                ps[:], lhsT=wqkv_h[:, kc, mt * 128:(mt + 1) * 128],
                                rhs=xf[:, kc, :],
                                start=(kc == 0), stop=(kc == KC - 1))
                        if mt == 0:
                            nc.scalar.copy(qT[:, lo:hi], ps[:])
                        else:
                            nc.scalar.copy(
                                kT[:].rearrange("p a b -> p (a b)")[:, lo:hi],
                                ps[:])
                    for j4 in range(4):
                        psv = vp.tile([128, 128], F32, tag="v")
                        nt = 4 * nch + j4
                        for kc in range(KC):
                            nc.tensor.matmul(psv[:], lhsT=xf[:, kc, j4 * 128:(j4 + 1) * 128],
                                             rhs=wqkv_h[:, kc, 256:384],
                                             start=(kc == 0), stop=(kc == KC - 1))
                        nc.vector.tensor_copy(v0[:, nt, 0:64], psv[:, 0:64])
                        nc.vector.tensor_copy(v1[:, nt, 0:64], psv[:, 64:128])

            # ---- block-mean similarity + top-k selection (f32) ----
            with tc.tile_pool(name="selps", bufs=1, space="PSUM") as sp, \
                 tc.tile_pool(name="selsb", bufs=2) as sb:
                qm_ps = sp.tile([128, NB], F32, tag="qm")
                km_ps = sp.tile([128, NB], F32, tag="km")
                for kc in range(KC):
                    nc.tensor.matmul(qm_ps[:], lhsT=wqk_f32[:, kc, 0:128],
                                     rhs=xm[:, kc, :], start=(kc == 0), stop=(kc == KC - 1))
                for kc in range(KC):
                    nc.tensor.matmul(km_ps[:], lhsT=wqk_f32[:, kc, 128:256],
                                     rhs=xm[:, kc, :], start=(kc == 0), stop=(kc == KC - 1))
                qm_sb = sb.tile([128, NB], F32, tag="qm")
                km_sb = sb.tile([128, NB], F32, tag="km")
                nc.scalar.copy(qm_sb[:], qm_ps[:])
                nc.scalar.copy(km_sb[:], km_ps[:])

                sim_ps = sp.tile([64, NB], F32, tag="sim")
                for h in range(HPC):
                    nc.tensor.matmul(sim_ps[h * 32:(h + 1) * 32, :],
                                     lhsT=qm_sb[h * 64:(h + 1) * 64, :],
                                     rhs=km_sb[h * 64:(h + 1) * 64, :],
                                     start=True, stop=True)
                sim2 = sb.tile([64, NB], F32, tag="sim2")
                nc.vector.tensor_copy(sim2[:], sim_ps[:])

                vals0 = sb.tile([64, 8], F32, tag="v0")
                idx0 = sb.tile([64, 8], U32, tag="i0")
                pun = sb.tile([64, NB], F32, tag="pun")
                vals1 = sb.tile([64, 8], F32, tag="v1")
                idx1 = sb.tile([64, 8], U32, tag="i1")
                nc.vector.max(vals0[:], sim2[:])
                nc.vector.max_index(idx0[:], vals0[:], sim2[:])
                nc.vector.match_replace(out=pun[:], in_to_replace=vals0[:],
                                        in_values=sim2[:], imm_value=-1e30)
                nc.vector.max(vals1[:], pun[:])
                nc.vector.max_index(idx1[:], vals1[:], pun[:])

                idxf = sb.tile([64, TOPK], F32, tag="idxf")
                nc.vector.tensor_copy(idxf[:, 0:8], idx0[:])
                nc.vector.tensor_copy(idxf[:, 8:16], idx1[:])

                selT_ps = sp.tile([TOPK, 64], F32, tag="selT")
                nc.tensor.transpose(selT_ps[:], idxf[:], id64[:])
                selT = sb.tile([TOPK, 64], F32, tag="selTsb")
                nc.vector.tensor_copy(selT[:], selT_ps[:])

                # replicate selT rows to all 16-partition groups via one matmul
                rep_ps = sp.tile([128, 64], F32, tag="rep")
                nc.tensor.matmul(rep_ps[:], lhsT=erep_sb[:], rhs=selT[:],
                                 start=True, stop=True)
                nc.vector.tensor_copy(kidx[0:64, :], rep_ps[0:64, 0:32])
                nc.vector.tensor_copy(kidx[64:128, :], rep_ps[64:128, 32:64])
                nc.vector.tensor_copy(vidx0[:], rep_ps[:, 0:32])
                nc.vector.tensor_copy(vidx1[:], rep_ps[:, 32:64])

            # ---- main loop: sparse attention + chunked projection partials ----
            with tc.tile_pool(name="gather", bufs=4) as gp, \
                 tc.tile_pool(name="escore", bufs=16) as ep, \
                 tc.tile_pool(name="sps", bufs=3, space="PSUM") as spp, \
                 tc.tile_pool(name="pjps", bufs=1, space="PSUM") as jpp, \
                 tc.tile_pool(name="ops", bufs=1, space="PSUM") as opp, \
                 tc.tile_pool(name="onp", bufs=3) as onp, \
                 tc.tile_pool(name="otsb", bufs=2) as otp, \
                 tc.tile_pool(name="posb", bufs=2) as pop:

                gathers = {}
                escores = {}
                ot_tiles = {}
                po_tiles = {}
                proj_q = []

                def emit_gathers(qb):
                    kg = gp.tile([128, TOPK, BLK], F16, tag="kg",
                                 name=f"kg_{qb}")
                    nc.gpsimd.ap_gather(kg[:], kT[:], kidx[:, qb:qb + 1],
                                        channels=128, num_elems=NB, d=BLK, num_idxs=TOPK)
                    vg0 = gp.tile([128, TOPK, 66], F16, tag="vg0",
                                  name=f"vg0_{qb}")
                    nc.gpsimd.ap_gather(vg0[:], v0[:], vidx0[:, qb:qb + 1],
                                        channels=128, num_elems=NB, d=66, num_idxs=TOPK)
                    vg1 = gp.tile([128, TOPK, 66], F16, tag="vg1",
                                  name=f"vg1_{qb}")
                    nc.gpsimd.ap_gather(vg1[:], v1[:], vidx1[:, qb:qb + 1],
                                        channels=128, num_elems=NB, d=66, num_idxs=TOPK)
                    gathers[qb] = (kg, vg0, vg1)

                def emit_scores_half(qb, half):
                    kg, _, _ = gathers[qb]
                    qcol = slice(qb * BLK, (qb + 1) * BLK)
                    s0 = spp.tile([128, 1024], F32, tag="s", name=f"s0_{qb}_{half}")
                    s1 = spp.tile([128, 1024], F32, tag="s", name=f"s1_{qb}_{half}")
                    for jj in range(8):
                        j = half * 8 + jj
                        nc.tensor.matmul(s0[:, jj * 128:(jj + 1) * 128],
                                         lhsT=kg[0:64, j, :], rhs=qT[0:64, qcol],
                                         start=True, stop=True)
                        nc.tensor.matmul(s1[:, jj * 128:(jj + 1) * 128],
                                         lhsT=kg[64:128, j, :], rhs=qT[64:128, qcol],
                                         start=True, stop=True)
                    e0 = ep.tile([128, 1024], F16, tag="e", name=f"e0_{qb}_{half}")
                    e1 = ep.tile([128, 1024], F16, tag="e", name=f"e1_{qb}_{half}")
                    nc.scalar.activation(e0[:], s0[:],
                                         mybir.ActivationFunctionType.Exp, scale=SCALE)
                    nc.scalar.activation(e1[:], s1[:],
                                         mybir.ActivationFunctionType.Exp, scale=SCALE)
                    if half == 0:
                        escores[qb] = [[e0, None], [e1, None]]
                    else:
                        escores[qb][0][1] = e0
                        escores[qb][1][1] = e1

                def emit_av(qb, h):
                    etiles = escores[qb]
                    _, vg0, vg1 = gathers[qb]
                    if h == 0:
                        o_ps = opp.tile([128, 2, 66], F32, tag="o",
                                        name=f"o_{qb}")
                        escores[qb].append(o_ps)
                    o_ps = escores[qb][2]
                    vg = vg0 if h == 0 else vg1
                    for j in range(TOPK):
                        nc.tensor.matmul(o_ps[:, h, 0:65],
                                         lhsT=etiles[h][j // 8][:, (j % 8) * 128:(j % 8 + 1) * 128],
                                         rhs=vg[:, j, 0:65],
                                         start=(j == 0), stop=(j == TOPK - 1))

                def emit_norm(qb):
                    o_ps = escores[qb][2]
                    onorm = onp.tile([128, 2 * D], F16, tag="onorm",
                                     name=f"on_{qb}")
                    for h in (0, 1):
                        rec = onp.tile([128, 1], F32, tag="rec", name=f"r_{qb}_{h}")
                        nc.vector.reciprocal(rec[:], o_ps[:, h, 64:65])
                        nc.vector.tensor_scalar(onorm[:, h * D:(h + 1) * D],
                                                o_ps[:, h, 0:D], rec[:], None,
                                                op0=mybir.AluOpType.mult)
                    nc.sync.dma_start(obounce.ap()[qb * BLK:(qb + 1) * BLK, :],
                                      onorm[:])
                    del escores[qb]
                    del gathers[qb]
                    if qb % CHQ == CHQ - 1:
                        c = qb // CHQ
                        ot = otp.tile([128, CHT], F16, tag="ot", name=f"ot_{c}")
                        nc.sync.dma_start_transpose(
                            ot[:], obounce.ap()[c * CHT:(c + 1) * CHT, :])
                        ot_tiles[c] = ot
                        po_tiles[c] = pop.tile([128, KC, CHT], F16, tag="po",
                                               name=f"po_{c}")
                        for m in range(KC):
                            proj_q.append((c, m))

                def emit_proj_some(k):
                    for _ in range(k):
                        if not proj_q:
                            return
                        c, m = proj_q.pop(0)
                        pj = jpp.tile([128, CHT], F32, tag="pj", name=f"pj_{c}_{m}")
                        nc.tensor.matmul(pj[:],
                                         lhsT=projW_h[:, m * 128:(m + 1) * 128],
                                         rhs=ot_tiles[c][:], start=True, stop=True)
                        nc.vector.tensor_scalar(po_tiles[c][:, m, :], pj[:],
                                                projb_sb[:, m:m + 1], None,
                                                op0=mybir.AluOpType.add)
                        if m == KC - 1:
                            nc.sync.dma_start(
                                out_ext.ap()[:, c * CHT:(c + 1) * CHT]
                                .rearrange("(a p) m -> p a m", p=128),
                                po_tiles.pop(c)[:])
                            ot_tiles.pop(c)

                # ---- software-pipelined main loop ----
                emit_gathers(0)
                emit_gathers(1)
                for qb in range(NB):
                    if qb + 2 < NB:
                        emit_gathers(qb + 2)
                    emit_proj_some(1)
                    emit_scores_half(qb, 0)
                    if qb > 0:
                        emit_av(qb - 1, 0)
                    emit_proj_some(1)
                    emit_scores_half(qb, 1)
                    if qb > 0:
                        emit_av(qb - 1, 1)
                        emit_norm(qb - 1)
                # epilogue
                emit_av(NB - 1, 0)
                emit_av(NB - 1, 1)
                emit_norm(NB - 1)
                emit_proj_some(len(proj_q))

    nc.compile()
    return nc


def _prep_inputs(x, qkv_w, proj_w, proj_b):
    x = np.asarray(x, dtype=np.float32)
    qkv_w = np.asarray(qkv_w, dtype=np.float32)
    proj_w = np.asarray(proj_w, dtype=np.float32)
    proj_b = np.asarray(proj_b, dtype=np.float32)

    xT = np.ascontiguousarray(x[0].T).astype(np.float16)   # [C, N] fp16
    ident64 = np.eye(64, dtype=np.float32)
    erep = (np.arange(128)[None, :] % 16 == np.arange(16)[:, None]).astype(np.float32)
    zero_b = np.zeros((128, 8), dtype=np.float32)
    in_maps = []
    for i in range(NCORES):
        h0 = HPC * i
        rows = []
        for part in range(3):                              # q, k, v row groups
            base = part * C + h0 * D
            rows.append(qkv_w[base:base + HPC * D, :])
        wqkv = np.concatenate(rows, axis=0)                # [384, C]
        wqkvT_np = np.ascontiguousarray(wqkv.T)            # [C, 384]
        cslice = slice(i * 2 * D, (i + 1) * 2 * D)
        in_maps.append({
            "xT": xT,
            "wqkvT": wqkvT_np.astype(np.float16),
            "wqkT32": np.ascontiguousarray(wqkvT_np[:, 0:256]),
            "projWT": np.ascontiguousarray(proj_w[:, cslice].T).astype(np.float16),
            "projb": (np.ascontiguousarray(proj_b.reshape(8, 128).T)
                      if i == 0 else zero_b),
            "ident64": ident64,
            "erep": erep,
        })
    return in_maps


def kernel(x, qkv_w, proj_w, proj_b, _trace=False):
    if "nc" not in _CACHE:
        _CACHE["nc"] = _build()
    nc = _CACHE["nc"]
    in_maps = _prep_inputs(x, qkv_w, proj_w, proj_b)
    res = run_bass_kernel_spmd(nc, in_maps, core_ids=list(range(NCORES)),
                               trace=_trace)
    outT = res.results[0]["out"].astype(np.float32)
    for i in range(1, NCORES):
        outT += res.results[i]["out"].astype(np.float32)
    out = np.ascontiguousarray(outT.T).reshape(1, N, C).astype(np.float32)
    if _trace:
        _CACHE["last_exec_time_ns"] = res.exec_time_ns
        _CACHE["last_results"] = res
    return out
